# revision 2
# baseline (speedup 1.0000x reference)
"""Trainium2 Bass kernel for nn_BinaryLinear (8192x4096 @ sign(4096x4096).T + BN + sign).

Math: out = sign((y - mean_b(y)) * rsqrt(var + eps) * gamma + beta), y = x @ sign(W).T + b.
With the reference's gamma == 1 (> 0) and beta == 0 the rsqrt/gamma factor is a positive
per-channel scale and beta vanishes, so out == sign(y - mean_b(y)); the bias b cancels in
y - mean, and mean is linear in x: y - mean = x @ Wb.T - (colmean(x) @ Wb.T).  The kernel
computes out = sign(x @ Wb.T - m) with m = colmean(x) @ Wb.T — no variance pass and no
second sweep over y.

Distribution: data-parallel over the batch dim, 1024 rows per NeuronCore.  Per-channel
column sums of x are AllReduced (16 KB) so every core subtracts the same global mean.

Precision: the matmul runs as two bf16 passes (x_hi + x_lo with x = x_hi + x_lo exact to
~2^-18 relative) accumulated in fp32 PSUM.  Wb = sign(W) is exactly representable in
bf16, so the result matches an fp32 matmul to ~1e-6 relative — only a handful of sign
flips at |y - mean| ~ 1e-4 remain vs the fp32 reference.

Per-core schedule: split x rows into bf16 hi/lo (DRAM bounce) while accumulating column
sums via ones-matmuls; AllReduce; then two batch-halves of 512 rows each: xbar-transposed
loads of x into [i-part, b] tiles, and per o-tile: binarize W rows on ACT (half 0 only,
cached in DRAM), xbar-transposed load of Wb, 64+64 accumulating matmuls + a tiny matmul
for m, Sign epilogue from PSUM on ACT, xbar-transpose back, cast to fp32, store.
"""
import sys

try:
    import concourse.bass as bass  # noqa: F401
except ImportError:
    sys.path.insert(0, "/opt/trn_rl_repo")

import numpy as np
import concourse.mybir as mybir
import concourse.tile as tile
from concourse import bacc
from concourse.bass_utils import run_bass_kernel_spmd

N_CORES = 8
B, D = 8192, 4096
BS = B // N_CORES          # 1024 batch rows per core
P = 128
NB = BS // P               # 8 batch tiles per core
NK = D // P                # 32 contraction tiles
NO = D // P                # 32 output-channel tiles
HB = BS // 2               # 512 rows per batch-half
F32, BF16 = mybir.dt.float32, mybir.dt.bfloat16

_CACHED_NC = None


def _build_nc():
    nc = bacc.Bacc("TRN2", target_bir_lowering=False, debug=False, num_devices=N_CORES)
    xs = nc.declare_dram_parameter("xs", [BS, D], F32, isOutput=False)
    W = nc.declare_dram_parameter("W", [D, D], F32, isOutput=False)
    out = nc.declare_dram_parameter("out", [BS, D], F32, isOutput=True)

    with tile.TileContext(nc) as tc:
        with (
            tc.tile_pool(name="const", bufs=1) as const,
            tc.tile_pool(name="xstage", bufs=2) as xstage,
            tc.tile_pool(name="xsplit", bufs=2) as xsplit,
            tc.tile_pool(name="xT", bufs=1) as xTp,
            tc.tile_pool(name="wstage", bufs=2) as wstage,
            tc.tile_pool(name="wsign", bufs=2) as wsign,
            tc.tile_pool(name="wT", bufs=3) as wTp,
            tc.tile_pool(name="epi", bufs=3) as epi,
            tc.tile_pool(name="blk", bufs=6) as blkp,
            tc.tile_pool(name="stats", bufs=1) as stats,
            tc.tile_pool(name="nm", bufs=2) as nmp,
            tc.tile_pool(name="ps", bufs=3, space="PSUM") as ps,
            tc.tile_pool(name="pcs", bufs=2, space="PSUM") as pcsp,
            tc.tile_pool(name="pm", bufs=2, space="PSUM") as pmp,
            tc.tile_pool(name="dram", bufs=1, space="DRAM") as dram,
            tc.tile_pool(name="wbdram", bufs=NO, space="DRAM") as wbdram,
        ):
            ones = const.tile([P, 1], BF16)
            nc.vector.memset(ones[:], 1.0)
            cs_sb = stats.tile([1, D], F32, tag="cs_sb")
            nc.vector.memset(cs_sb[:], 0.0)

            xhi_d = dram.tile([BS, D], BF16, tag="xhi")
            xlo_d = dram.tile([BS, D], BF16, tag="xlo")
            cs_in = dram.tile([1, D], F32, tag="cs_in")
            cs_out = dram.tile([1, D], F32, tag="cs_out")

            # ---- Phase 1: split x into bf16 hi/lo (DRAM bounce) + colsum partials.
            # Processed in [128, 2048] pieces to keep SBUF staging small.
            HD = D // 2
            for bt in range(NB):
                for hh in range(2):
                    cs, ce = hh * HD, (hh + 1) * HD
                    xf = xstage.tile([P, HD], F32, tag="xf")
                    nc.sync.dma_start(xf[:], xs[bt * P:(bt + 1) * P, cs:ce])
                    xh = xsplit.tile([P, HD], BF16, tag="xh")
                    nc.vector.tensor_copy(xh[:], xf[:])
                    # residual in place: xf <- xf - xh
                    nc.vector.tensor_sub(xf[:], xf[:], xh[:])
                    xl = xsplit.tile([P, HD], BF16, tag="xl")
                    nc.vector.tensor_copy(xl[:], xf[:])
                    nc.sync.dma_start(xhi_d[bt * P:(bt + 1) * P, cs:ce], xh[:])
                    nc.sync.dma_start(xlo_d[bt * P:(bt + 1) * P, cs:ce], xl[:])
                    # colsum of (hi + lo) for this piece, accumulated into cs_sb
                    for c in range(HD // 512):
                        g0 = cs + c * 512
                        pcs = pcsp.tile([1, 512], F32, tag="pcs")
                        nc.tensor.matmul(pcs[:], ones[:], xh[:, c * 512:(c + 1) * 512],
                                         start=True, stop=False)
                        nc.tensor.matmul(pcs[:], ones[:], xl[:, c * 512:(c + 1) * 512],
                                         start=False, stop=True)
                        nc.vector.tensor_add(cs_sb[0:1, g0:g0 + 512],
                                             cs_sb[0:1, g0:g0 + 512], pcs[:])

            # ---- Phase 1b: AllReduce colsum; build -xbar hi/lo in [i-part, k] layout
            nc.sync.dma_start(cs_in[:], cs_sb[:])
            nc.gpsimd.collective_compute(
                "AllReduce", mybir.AluOpType.add,
                replica_groups=[list(range(N_CORES))],
                ins=[cs_in.opt()], outs=[cs_out.opt()],
            )
            csT = stats.tile([P, NK], F32, tag="csT")
            nc.gpsimd.dma_start(csT[:], cs_out[0].rearrange("(t p) -> p t", p=P))
            nxb = stats.tile([P, NK], F32, tag="nxb")
            nc.vector.tensor_scalar_mul(nxb[:], csT[:], -1.0 / B)
            nxh = stats.tile([P, NK], BF16, tag="nxh")
            nc.vector.tensor_copy(nxh[:], nxb[:])
            nc.vector.tensor_sub(nxb[:], nxb[:], nxh[:])
            nxl = stats.tile([P, NK], BF16, tag="nxl")
            nc.vector.tensor_copy(nxl[:], nxb[:])
            nxhl = stats.tile([P, NK, 2], BF16, tag="nxhl")
            nc.vector.tensor_copy(nxhl[:, :, 0], nxh[:])
            nc.vector.tensor_copy(nxhl[:, :, 1], nxl[:])

            wb_tiles = [None] * NO
            negm_tiles = [None] * NO
            negm_pool = stats  # [P, 1] f32 per o-tile, kept for the second half

            for half in range(2):
                r0 = half * HB
                # ---- Phase 2: transposed loads of this half of x into [i, b] tiles
                xT_hi, xT_lo = [], []
                for k in range(NK):
                    th = xTp.tile([P, HB], BF16, tag=f"xh{k}")
                    nc.sync.dma_start_transpose(
                        th[:], xhi_d[r0:r0 + HB, k * P:(k + 1) * P])
                    tl = xTp.tile([P, HB], BF16, tag=f"xl{k}")
                    nc.sync.dma_start_transpose(
                        tl[:], xlo_d[r0:r0 + HB, k * P:(k + 1) * P])
                    xT_hi.append(th)
                    xT_lo.append(tl)

                # ---- Phase 3: per o-tile
                for o in range(NO):
                    if half == 0:
                        wb_d = wbdram.tile([P, D], BF16, tag="wb")
                        wb_tiles[o] = wb_d
                        for h in range(2):
                            wf = wstage.tile([P, HD], F32, tag="wf")
                            nc.sync.dma_start(
                                wf[:], W[o * P:(o + 1) * P, h * HD:(h + 1) * HD])
                            wsg = wsign.tile([P, HD], BF16, tag="ws")
                            nc.scalar.sign(wsg[:], wf[:])
                            nc.sync.dma_start(wb_d[:, h * HD:(h + 1) * HD], wsg[:])
                    else:
                        wb_d = wb_tiles[o]
                    wT = wTp.tile([P, NK, P], BF16, tag="wT")
                    nc.sync.dma_start_transpose(wT[:], wb_d[:, :])

                    psum = ps.tile([P, HB], F32, tag="acc")
                    if half == 0:
                        pm = pmp.tile([P, 2], F32, tag="pm")
                    for k in range(NK):
                        lhsT = wT[:, k, :]
                        nc.tensor.matmul(psum[:], lhsT, xT_hi[k][:],
                                         start=(k == 0), stop=False)
                        nc.tensor.matmul(psum[:], lhsT, xT_lo[k][:],
                                         start=False, stop=(k == NK - 1))
                        if half == 0:
                            nc.tensor.matmul(pm[:], lhsT, nxhl[:, k, :],
                                             start=(k == 0), stop=(k == NK - 1))
                    if half == 0:
                        pmc = nmp.tile([P, 2], F32, tag="pmc")
                        nc.vector.tensor_copy(pmc[:], pm[:])
                        negm = negm_pool.tile([P, 1], F32, tag=f"negm{o}")
                        nc.vector.tensor_add(negm[:], pmc[:, 0:1], pmc[:, 1:2])
                        negm_tiles[o] = negm
                    else:
                        negm = negm_tiles[o]
                    ys = epi.tile([P, HB], BF16, tag="ys")
                    nc.scalar.activation(out=ys[:], in_=psum[:],
                                         func=mybir.ActivationFunctionType.Sign,
                                         bias=negm[:], scale=1.0)
                    for bt in range(HB // P):
                        blk = blkp.tile([P, P], BF16, tag="blk")
                        nc.sync.dma_start_transpose(blk[:], ys[:, bt * P:(bt + 1) * P])
                        blk32 = blkp.tile([P, P], F32, tag="blk32")
                        nc.vector.tensor_copy(blk32[:], blk[:])
                        nc.sync.dma_start(
                            out[r0 + bt * P:r0 + (bt + 1) * P, o * P:(o + 1) * P],
                            blk32[:])

    nc.finalize()
    return nc


def _get_nc():
    global _CACHED_NC
    if _CACHED_NC is None:
        _CACHED_NC = _build_nc()
    return _CACHED_NC


def _run(x, W, **kw):
    nc = _get_nc()
    in_maps = [{"xs": x[c * BS:(c + 1) * BS], "W": W} for c in range(N_CORES)]
    res = run_bass_kernel_spmd(nc, in_maps, list(range(N_CORES)), **kw)
    full = np.concatenate([res.results[c]["out"] for c in range(N_CORES)], axis=0)
    return full, res


def kernel(x, W, b, gamma, beta):
    x = np.ascontiguousarray(x, dtype=np.float32)
    W = np.ascontiguousarray(W, dtype=np.float32)
    assert x.shape == (B, D) and W.shape == (D, D)
    if not (np.all(np.asarray(gamma) > 0) and np.all(np.asarray(beta) == 0)):
        # The sign(y - mean) reduction needs gamma > 0 and beta == 0 (always true for
        # this problem's inputs).  Otherwise fall back to a host computation.
        Wb = np.sign(W)
        y = x @ Wb.T + np.asarray(b, np.float32)
        mean = y.mean(0)
        var = ((y - mean) ** 2).mean(0)
        yn = (y - mean) / np.sqrt(var + 1e-5) * np.asarray(gamma) + np.asarray(beta)
        return np.sign(yn).astype(np.float32)
    full, _ = _run(x, W)
    return full.astype(np.float32, copy=False)


# revision 5
# speedup vs baseline: 1.0390x; 1.0390x over previous
"""Trainium2 Bass kernel for nn_BinaryLinear (8192x4096 @ sign(4096x4096).T + BN + sign).

Math: out = sign((y - mean_b(y)) * rsqrt(var + eps) * gamma + beta), y = x @ sign(W).T + b.
With the reference's gamma == 1 (> 0) and beta == 0 the rsqrt/gamma factor is a positive
per-channel scale and beta vanishes, so out == sign(y - mean_b(y)); the bias b cancels in
y - mean, and mean is linear in x: y - mean = x @ Wb.T - (colmean(x) @ Wb.T).  The kernel
computes out = sign(x @ Wb.T - m) with m = colmean(x) @ Wb.T — no variance pass and no
second sweep over y.

Distribution: data-parallel over the batch dim, 1024 rows per NeuronCore.  Per-channel
column sums of x are AllReduced (16 KB) so every core subtracts the same global mean.

Precision: the matmul runs as two bf16 passes (x_hi + x_lo with x = x_hi + x_lo exact to
~2^-18 relative) accumulated in fp32 PSUM.  Wb = sign(W) is exactly representable in
bf16, so the result matches an fp32 matmul to ~1e-6 relative — only a handful of sign
flips at |y - mean| ~ 1e-4 remain vs the fp32 reference.

Per-core schedule: split x rows into bf16 hi/lo (DRAM bounce, one tile per batch-half)
while ones-matmuls accumulate column sums; AllReduce; then two batch-halves of 512 rows:
xbar-transposed loads of x into [i-part, b] tiles, and per o-tile: binarize W rows on ACT
(half 0 only, cached in DRAM), xbar-transposed load of Wb, 64 accumulating matmuls plus a
tiny matmul for m, Sign epilogue from PSUM on ACT, one xbar transpose back, one fp32
cast, one strided store.  DMA issue is spread over SP (xbar transposes, x loads), ACT
(epilogue) and SWDGE (W pipeline, stores) to keep the wT prefetch unblocked.
"""
import sys

try:
    import concourse.bass as bass  # noqa: F401
except ImportError:
    sys.path.insert(0, "/opt/trn_rl_repo")

import numpy as np
import concourse.mybir as mybir
import concourse.tile as tile
from concourse import bacc
from concourse.bass_utils import run_bass_kernel_spmd

N_CORES = 8
B, D = 8192, 4096
BS = B // N_CORES          # 1024 batch rows per core
P = 128
NB = BS // P               # 8 batch tiles per core
NK = D // P                # 32 contraction tiles
NO = D // P                # 32 output-channel tiles
HB = BS // 2               # 512 rows per batch-half
HD = D // 2
F32, BF16 = mybir.dt.float32, mybir.dt.bfloat16

_CACHED_NC = None


def _build_nc():
    nc = bacc.Bacc("TRN2", target_bir_lowering=False, debug=False, num_devices=N_CORES)
    xs = nc.declare_dram_parameter("xs", [BS, D], F32, isOutput=False)
    W = nc.declare_dram_parameter("W", [D, D], F32, isOutput=False)
    out = nc.declare_dram_parameter("out", [BS, D], F32, isOutput=True)

    with tile.TileContext(nc) as tc:
        with (
            tc.tile_pool(name="const", bufs=1) as const,
            tc.tile_pool(name="xstage", bufs=2) as xstage,
            tc.tile_pool(name="xsplit", bufs=2) as xsplit,
            tc.tile_pool(name="xT", bufs=1) as xTp,
            tc.tile_pool(name="wstage", bufs=2) as wstage,
            tc.tile_pool(name="wsign", bufs=2) as wsign,
            tc.tile_pool(name="wT", bufs=3) as wTp,
            tc.tile_pool(name="epi", bufs=3) as epi,
            tc.tile_pool(name="stats", bufs=1) as stats,
            tc.tile_pool(name="nm", bufs=2) as nmp,
            tc.tile_pool(name="ps", bufs=3, space="PSUM") as ps,
            tc.tile_pool(name="pcs", bufs=2, space="PSUM") as pcsp,
            tc.tile_pool(name="pm", bufs=2, space="PSUM") as pmp,
            tc.tile_pool(name="dram", bufs=1, space="DRAM") as dram,
            tc.tile_pool(name="wbdram", bufs=NO, space="DRAM") as wbdram,
        ):
            ones = const.tile([P, 1], BF16)
            nc.vector.memset(ones[:], 1.0)

            # per-half DRAM bounce tiles so each half's transposed loads only
            # depend on that half's stores
            xhi_d = [dram.tile([HB, D], BF16, tag=f"xhi{h}", name=f"xhi{h}") for h in range(2)]
            xlo_d = [dram.tile([HB, D], BF16, tag=f"xlo{h}", name=f"xlo{h}") for h in range(2)]
            cs_in = dram.tile([1, D], F32, tag="cs_in")
            cs_out = dram.tile([1, D], F32, tag="cs_out")

            # 8 independent colsum chunk accumulators (no false serialization)
            NCH = D // 512
            cs_chunks = []
            for c in range(NCH):
                t = stats.tile([1, 512], F32, tag=f"cs{c}", name=f"cs{c}")
                nc.vector.memset(t[:], 0.0)
                cs_chunks.append(t)

            # ---- Phase 1: split x into bf16 hi/lo (DRAM bounce) + colsum partials
            for bt in range(NB):
                half, row = bt // (NB // 2), (bt % (NB // 2)) * P
                for hh in range(2):
                    c0 = hh * HD
                    xf = xstage.tile([P, HD], F32, tag="xf")
                    nc.sync.dma_start(xf[:], xs[bt * P:(bt + 1) * P, c0:c0 + HD])
                    xh = xsplit.tile([P, HD], BF16, tag="xh")
                    nc.vector.tensor_copy(xh[:], xf[:])
                    nc.vector.tensor_sub(xf[:], xf[:], xh[:])   # residual in place
                    xl = xsplit.tile([P, HD], BF16, tag="xl")
                    nc.vector.tensor_copy(xl[:], xf[:])
                    nc.gpsimd.dma_start(xhi_d[half][row:row + P, c0:c0 + HD], xh[:])
                    nc.gpsimd.dma_start(xlo_d[half][row:row + P, c0:c0 + HD], xl[:])
                    for c in range(HD // 512):
                        g = hh * (HD // 512) + c
                        pcs = pcsp.tile([1, 512], F32, tag="pcs")
                        nc.tensor.matmul(pcs[:], ones[:], xh[:, c * 512:(c + 1) * 512],
                                         start=True, stop=False)
                        nc.tensor.matmul(pcs[:], ones[:], xl[:, c * 512:(c + 1) * 512],
                                         start=False, stop=True)
                        nc.vector.tensor_add(cs_chunks[g][:], cs_chunks[g][:], pcs[:])

            # ---- Phase 1b: AllReduce colsum; build -xbar hi/lo in [i-part, k] layout
            for c in range(NCH):
                nc.gpsimd.dma_start(cs_in[0:1, c * 512:(c + 1) * 512], cs_chunks[c][:])
            nc.gpsimd.collective_compute(
                "AllReduce", mybir.AluOpType.add,
                replica_groups=[list(range(N_CORES))],
                ins=[cs_in.opt()], outs=[cs_out.opt()],
            )
            csT = stats.tile([P, NK], F32, tag="csT")
            nc.gpsimd.dma_start(csT[:], cs_out[0].rearrange("(t p) -> p t", p=P))
            nxb = stats.tile([P, NK], F32, tag="nxb")
            nc.vector.tensor_scalar_mul(nxb[:], csT[:], -1.0 / B)
            nxh = stats.tile([P, NK], BF16, tag="nxh")
            nc.vector.tensor_copy(nxh[:], nxb[:])
            nc.vector.tensor_sub(nxb[:], nxb[:], nxh[:])
            nxl = stats.tile([P, NK], BF16, tag="nxl")
            nc.vector.tensor_copy(nxl[:], nxb[:])
            nxhl = stats.tile([P, NK, 2], BF16, tag="nxhl")
            nc.vector.tensor_copy(nxhl[:, :, 0], nxh[:])
            nc.vector.tensor_copy(nxhl[:, :, 1], nxl[:])

            wb_tiles = [None] * NO
            negm_tiles = [None] * NO

            for half in range(2):
                r0 = half * HB
                # ---- Phase 2: transposed loads of this half of x into [i, b] tiles
                xT_hi, xT_lo = [], []
                for k in range(NK):
                    th = xTp.tile([P, HB], BF16, tag=f"xh{k}")
                    nc.sync.dma_start_transpose(
                        th[:], xhi_d[half][:, k * P:(k + 1) * P])
                    tl = xTp.tile([P, HB], BF16, tag=f"xl{k}")
                    nc.sync.dma_start_transpose(
                        tl[:], xlo_d[half][:, k * P:(k + 1) * P])
                    xT_hi.append(th)
                    xT_lo.append(tl)

                # ---- Phase 3: per o-tile
                for o in range(NO):
                    if half == 0:
                        wb_d = wbdram.tile([P, D], BF16, tag="wb")
                        wb_tiles[o] = wb_d
                        for h in range(2):
                            wf = wstage.tile([P, HD], F32, tag="wf")
                            nc.gpsimd.dma_start(
                                wf[:], W[o * P:(o + 1) * P, h * HD:(h + 1) * HD])
                            wsg = wsign.tile([P, HD], BF16, tag="ws")
                            nc.scalar.sign(wsg[:], wf[:])
                            nc.gpsimd.dma_start(wb_d[:, h * HD:(h + 1) * HD], wsg[:])
                    else:
                        wb_d = wb_tiles[o]
                    wT = wTp.tile([P, NK, P], BF16, tag="wT")
                    nc.sync.dma_start_transpose(wT[:], wb_d[:, :])

                    psum = ps.tile([P, HB], F32, tag="acc")
                    if half == 0:
                        pm = pmp.tile([P, 2], F32, tag="pm")
                    for k in range(NK):
                        lhsT = wT[:, k, :]
                        nc.tensor.matmul(psum[:], lhsT, xT_hi[k][:],
                                         start=(k == 0), stop=False)
                        nc.tensor.matmul(psum[:], lhsT, xT_lo[k][:],
                                         start=False, stop=(k == NK - 1))
                        if half == 0:
                            nc.tensor.matmul(pm[:], lhsT, nxhl[:, k, :],
                                             start=(k == 0), stop=(k == NK - 1))
                    if half == 0:
                        pmc = nmp.tile([P, 2], F32, tag="pmc")
                        nc.vector.tensor_copy(pmc[:], pm[:])
                        negm = stats.tile([P, 1], F32, tag=f"negm{o}")
                        nc.vector.tensor_add(negm[:], pmc[:, 0:1], pmc[:, 1:2])
                        negm_tiles[o] = negm
                    else:
                        negm = negm_tiles[o]
                    ys = epi.tile([P, HB], BF16, tag="ys")
                    nc.scalar.activation(out=ys[:], in_=psum[:],
                                         func=mybir.ActivationFunctionType.Sign,
                                         bias=negm[:], scale=1.0)
                    # one transposed copy back to [b, o] blocks, one cast, one store
                    ysT = epi.tile([P, HB // P, P], BF16, tag="ysT")
                    nc.sync.dma_start_transpose(ysT[:], ys[:])
                    ysT32 = epi.tile([P, HB // P, P], F32, tag="ysT32")
                    nc.vector.tensor_copy(ysT32[:], ysT[:])
                    nc.sync.dma_start(
                        out[r0:r0 + HB, o * P:(o + 1) * P].rearrange(
                            "(t p) j -> p t j", p=P),
                        ysT32[:])

    nc.finalize()
    return nc


def _get_nc():
    global _CACHED_NC
    if _CACHED_NC is None:
        _CACHED_NC = _build_nc()
    return _CACHED_NC


def _run(x, W, **kw):
    nc = _get_nc()
    in_maps = [{"xs": x[c * BS:(c + 1) * BS], "W": W} for c in range(N_CORES)]
    res = run_bass_kernel_spmd(nc, in_maps, list(range(N_CORES)), **kw)
    full = np.concatenate([res.results[c]["out"] for c in range(N_CORES)], axis=0)
    return full, res


def kernel(x, W, b, gamma, beta):
    x = np.ascontiguousarray(x, dtype=np.float32)
    W = np.ascontiguousarray(W, dtype=np.float32)
    assert x.shape == (B, D) and W.shape == (D, D)
    if not (np.all(np.asarray(gamma) > 0) and np.all(np.asarray(beta) == 0)):
        # The sign(y - mean) reduction needs gamma > 0 and beta == 0 (always true for
        # this problem's inputs).  Otherwise fall back to a host computation.
        Wb = np.sign(W)
        y = x @ Wb.T + np.asarray(b, np.float32)
        mean = y.mean(0)
        var = ((y - mean) ** 2).mean(0)
        yn = (y - mean) / np.sqrt(var + 1e-5) * np.asarray(gamma) + np.asarray(beta)
        return np.sign(yn).astype(np.float32)
    full, _ = _run(x, W)
    return full.astype(np.float32, copy=False)
